# revision 1
# baseline (speedup 1.0000x reference)
"""KMeans min-distance loss kernel for Trainium2 (8 NeuronCores, SPMD).

Problem: features [262144, 128] f32, centers [256, 128] f32.
  d2[n,k] = ||f_n||^2 + ||c_k||^2 - 2 f_n.c_k ; out = mean_n sqrt(min_k d2)

Sharding: data-parallel over N (32768 rows per core), centers replicated.
Each core returns [128] partial sums of min-distances; host reduces.

Per-core pipeline (bf16 compute, f32 accumulate):
  - SWDGE cast-DMA 1MB groups: f32 dram -> bf16 sbuf [128p, 16, 128]
  - PE transpose (bf16) chunks -> featT, batches of 4 per PSUM bank
  - ACT evacuates PSUM -> SBUF featT
  - PE: rank-1 fp16 matmul preloads centered ||c||^2 into PSUM, then
    bf16 cross matmuls accumulate -2 f.c  -> [128n, 4, 256k]
  - DVE segmented tensor_reduce min over k -> m[:, 4]
  - f2 = sum(f^2): alternates DVE scalar_tensor_tensor / ACT Square+accum
  - tail: sqrt(m + f2 + mean_c2) with ACT accum -> [128] sums -> DMA out
"""

import sys

for p in ("/opt/trn_rl_repo", "/opt/trn_rl_repo/concourse"):
    if p not in sys.path:
        sys.path.insert(0, p)

import numpy as np

N_TOTAL = 262144
K = 256
D = 128
N_CORES = 8
N_PER_CORE = N_TOTAL // N_CORES  # 32768
P = 128
CHUNKS = N_PER_CORE // P         # 256 chunks of 128 rows
G = 16                           # chunks per DMA group (1 MB f32 read)
GROUPS = CHUNKS // G             # 16
TG = 4                           # chunks per transpose/psum/reduce batch

_compiled = None


def _build(repeat: int = 1):
    import concourse.bass as bass
    import concourse.bacc as bacc
    import concourse.tile as tile
    from concourse import mybir

    f32 = mybir.dt.float32
    bf16 = mybir.dt.bfloat16
    fp16 = mybir.dt.float16
    Alu = mybir.AluOpType
    Act = mybir.ActivationFunctionType

    nc = bacc.Bacc(
        "TRN2", target_bir_lowering=False, debug=False, num_devices=N_CORES
    )

    feats = nc.dram_tensor("features", [N_PER_CORE, D], f32, kind="ExternalInput").ap()
    ctneg2 = nc.dram_tensor("ctneg2", [D, K], bf16, kind="ExternalInput").ap()
    c2q = nc.dram_tensor("c2q", [1, TG * K], fp16, kind="ExternalInput").ap()
    ones = nc.dram_tensor("ones", [1, P], fp16, kind="ExternalInput").ap()
    ident = nc.dram_tensor("ident", [P, P], bf16, kind="ExternalInput").ap()
    c2mean = nc.dram_tensor("c2mean", [P, 1], f32, kind="ExternalInput").ap()
    out = nc.dram_tensor("out", [P, 1], f32, kind="ExternalOutput").ap()

    with tile.TileContext(nc) as tc:
        with (
            tc.tile_pool(name="consts", bufs=1) as consts,
            tc.tile_pool(name="featg", bufs=3) as featg_pool,
            tc.tile_pool(name="featT", bufs=4) as featT_pool,
            tc.tile_pool(name="dumps", bufs=2) as dumps,
            tc.tile_pool(name="coll", bufs=1) as coll,
            tc.tile_pool(name="ptrans", bufs=2, space="PSUM") as ptrans_pool,
            tc.tile_pool(name="pcross", bufs=3, space="PSUM") as pcross_pool,
        ):
            ct_s = consts.tile([D, K], bf16)
            nc.sync.dma_start(ct_s[:], ctneg2)
            c2q_s = consts.tile([1, TG * K], fp16)
            nc.sync.dma_start(c2q_s[:], c2q)
            ones_s = consts.tile([1, P], fp16)
            nc.sync.dma_start(ones_s[:], ones)
            id_s = consts.tile([P, P], bf16)
            nc.sync.dma_start(id_s[:], ident)
            c2m_s = consts.tile([P, 1], f32)
            nc.sync.dma_start(c2m_s[:], c2mean)

            m_coll = coll.tile([P, CHUNKS], f32)
            f2_coll = coll.tile([P, CHUNKS], f32)

            # features viewed as [group, partition, chunk-in-group, d].
            # Partition p takes G consecutive rows (one 8KB contiguous
            # descriptor per partition); chunk->row mapping is permuted,
            # which the order-invariant sum tolerates.
            fview = feats.rearrange("(g p c) d -> g p c d", p=P, c=G)

            for g in range(GROUPS * repeat):
                g = g % GROUPS
                fg = featg_pool.tile([P, G, D], bf16)
                nc.gpsimd.dma_start(fg[:], fview[g])  # SWDGE cast f32->bf16

                for cb in range(G // TG):
                    pt = ptrans_pool.tile([D, TG * P], bf16)
                    for j in range(TG):
                        c = cb * TG + j
                        nc.tensor.transpose(
                            pt[:, bass.ts(j, P)], fg[:, c, :], id_s[:]
                        )
                    fT = featT_pool.tile([D, TG * P], bf16)
                    nc.scalar.copy(fT[:], pt[:])

                    px4 = pcross_pool.tile([P, TG, K], f32)
                    px4f = px4[:].rearrange("p c k -> p (c k)")
                    for h in range(2):
                        nc.tensor.matmul(
                            px4f[:, bass.ts(h, TG * K // 2)],
                            ones_s[:],
                            c2q_s[:, bass.ts(h, TG * K // 2)],
                            start=True, stop=False, skip_group_check=True,
                        )
                    for j in range(TG):
                        c = cb * TG + j
                        i = g * G + c
                        # f2: alternate DVE / ACT to balance engines
                        if j % 2 == 0:
                            d128 = dumps.tile([P, D], bf16, tag="d128")
                            nc.vector.scalar_tensor_tensor(
                                out=d128[:],
                                in0=fg[:, c, :],
                                scalar=1.0,
                                in1=fg[:, c, :],
                                op0=Alu.mult,
                                op1=Alu.mult,
                                accum_out=f2_coll[:, i : i + 1],
                            )
                        else:
                            dA = dumps.tile([P, D], bf16, tag="dA")
                            nc.scalar.activation(
                                dA[:], fg[:, c, :], Act.Square,
                                accum_out=f2_coll[:, i : i + 1],
                            )
                        nc.tensor.matmul(
                            px4[:, j, :], fT[:, bass.ts(j, P)], ct_s[:],
                            start=False, stop=(j == TG - 1),
                            skip_group_check=True,
                        )
                    ib = g * G + cb * TG
                    nc.vector.tensor_reduce(
                        out=m_coll[:, ib : ib + TG],
                        in_=px4[:],
                        axis=mybir.AxisListType.X,
                        op=Alu.min,
                    )

            # tail: sums[p] = sum_i sqrt(m[p,i] + f2[p,i] + c2mean)
            d2t = coll.tile([P, CHUNKS], f32)
            nc.vector.tensor_add(d2t[:], m_coll[:], f2_coll[:])
            dist = coll.tile([P, CHUNKS], f32)
            sums = coll.tile([P, 1], f32)
            nc.scalar.activation(
                dist[:], d2t[:], Act.Sqrt, bias=c2m_s[:], accum_out=sums[:]
            )
            nc.sync.dma_start(out, sums[:])

    nc.compile()
    return nc


def _get_compiled():
    global _compiled
    if _compiled is None:
        _compiled = _build()
    return _compiled


def _make_aux(centers: np.ndarray):
    import ml_dtypes

    cen_bf = centers.astype(ml_dtypes.bfloat16)
    ctneg2 = np.ascontiguousarray(
        (-2.0 * cen_bf.astype(np.float32).T)
    ).astype(ml_dtypes.bfloat16)                                   # [D, K]
    c2 = (cen_bf.astype(np.float64) ** 2).sum(axis=1)              # [K]
    c2m = float(c2.mean())
    c2c = (c2 - c2m).astype(np.float16)
    c2q = np.ascontiguousarray(np.tile(c2c[None, :], (1, TG)))     # [1, TG*K]
    ones = np.ones((1, P), dtype=np.float16)
    ident = np.eye(P, dtype=ml_dtypes.bfloat16)
    c2mean = np.full((P, 1), c2m, dtype=np.float32)
    return ctneg2, c2q, ones, ident, c2mean


def _make_in_maps(features: np.ndarray, centers: np.ndarray):
    ctneg2, c2q, ones, ident, c2mean = _make_aux(centers)
    return [
        {
            "features": features[c * N_PER_CORE : (c + 1) * N_PER_CORE],
            "ctneg2": ctneg2,
            "c2q": c2q,
            "ones": ones,
            "ident": ident,
            "c2mean": c2mean,
        }
        for c in range(N_CORES)
    ]


def kernel(features: np.ndarray, centers: np.ndarray) -> np.ndarray:
    features = np.ascontiguousarray(np.asarray(features, dtype=np.float32))
    centers = np.ascontiguousarray(np.asarray(centers, dtype=np.float32))
    assert features.shape == (N_TOTAL, D) and centers.shape == (K, D)

    from concourse.bass_utils import run_bass_kernel_spmd

    nc = _get_compiled()
    in_maps = _make_in_maps(features, centers)
    res = run_bass_kernel_spmd(nc, in_maps, list(range(N_CORES)))
    total = 0.0
    for r in res.results:
        total += np.sum(r["out"].astype(np.float64))
    return np.float32(total / N_TOTAL)


if __name__ == "__main__":
    rng = np.random.default_rng(0)
    f = rng.standard_normal((N_TOTAL, D), dtype=np.float32)
    c = rng.standard_normal((K, D), dtype=np.float32)
    print(kernel(f, c))



# revision 2
# speedup vs baseline: 1.8237x; 1.8237x over previous
"""KMeans min-distance loss kernel for Trainium2 (8 NeuronCores, SPMD).

Problem: features [262144, 128] f32, centers [256, 128] f32.
  d2[n,k] = ||f_n||^2 + ||c_k||^2 - 2 f_n.c_k ; out = mean_n sqrt(min_k d2)

Sharding: data-parallel over N (32768 rows per core), centers replicated.
Each core returns [128] partial sums of min-distances; host reduces.

Per-core pipeline (fp8 DoubleRow matmul carries cross + f2 + c2):
  - SWDGE cast-DMA 1MB groups: f32 dram -> bf16 sbuf [128p, 16, 128]
  - PE transpose (bf16) chunks -> PSUM, batches of 8 per PSUM bank
  - ACT evacuates PSUM -> fT fp8: copy (features) + Square (squares)
  - one fp8 DoubleRow matmul per chunk, contraction 256:
      rows   0-127: f[n,d] x -2c[k,d]
      rows 128-253: f[n,d]^2 (d<126) x 1.0          (-> ||f||^2 partial)
      rows 254-255: 1.0 x c2a[k], 1.0 x c2b[k]      (-> centered ||c||^2,
                                                     fp8 error-feedback pair)
  - DVE segmented tensor_reduce min over k -> m[:, 4]
  - dims 126/127 squares via batched ACT Square from the [n,d] tile
  - tail: sqrt(m + sq126 + sq127 + mean_c2) with ACT accum -> [128] sums
"""

import sys

for p in ("/opt/trn_rl_repo", "/opt/trn_rl_repo/concourse"):
    if p not in sys.path:
        sys.path.insert(0, p)

import numpy as np

N_TOTAL = 262144
K = 256
D = 128
N_CORES = 8
N_PER_CORE = N_TOTAL // N_CORES  # 32768
P = 128
CHUNKS = N_PER_CORE // P         # 256 chunks of 128 rows
G = 16                           # chunks per DMA group (1 MB f32 read)
GROUPS = CHUNKS // G             # 16
TB = 8                           # chunks per transpose/evac batch
TG = 4                           # chunks per cross/reduce batch

_compiled = None


def _build():
    import concourse.bass as bass
    import concourse.bacc as bacc
    import concourse.tile as tile
    from concourse import mybir

    f32 = mybir.dt.float32
    bf16 = mybir.dt.bfloat16
    fp8 = mybir.dt.float8e4
    Alu = mybir.AluOpType
    Act = mybir.ActivationFunctionType

    nc = bacc.Bacc(
        "TRN2", target_bir_lowering=False, debug=False, num_devices=N_CORES
    )

    feats = nc.dram_tensor("features", [N_PER_CORE, D], f32, kind="ExternalInput").ap()
    ctp = nc.dram_tensor("ctp", [P, 2 * K], fp8, kind="ExternalInput").ap()
    cones = nc.dram_tensor("cones", [2, TB * P], fp8, kind="ExternalInput").ap()
    ident = nc.dram_tensor("ident", [P, P], bf16, kind="ExternalInput").ap()
    c2mean = nc.dram_tensor("c2mean", [P, 1], f32, kind="ExternalInput").ap()
    out = nc.dram_tensor("out", [P, 1], f32, kind="ExternalOutput").ap()

    with tile.TileContext(nc) as tc:
        with (
            tc.tile_pool(name="consts", bufs=1) as consts,
            tc.tile_pool(name="featg", bufs=3) as featg_pool,
            tc.tile_pool(name="coll", bufs=1) as coll,
            tc.tile_pool(name="ptrans", bufs=2, space="PSUM") as ptrans_pool,
            tc.tile_pool(name="pcross", bufs=3, space="PSUM") as pcross_pool,
        ):
            ct_s = consts.tile([P, 2 * K], fp8)
            nc.sync.dma_start(ct_s[:], ctp)
            ct_ap = ct_s[:].rearrange("p (t k) -> p t k", t=2)
            id_s = consts.tile([P, P], bf16)
            nc.sync.dma_start(id_s[:], ident)
            c2m_s = consts.tile([P, 1], f32)
            nc.sync.dma_start(c2m_s[:], c2mean)

            # two manually alternated fT buffers, each holding TB chunk
            # slots of [2 ktiles x 128 cols] fp8.  Slice-1 partitions
            # 126/127 hold the constant 1.0 rows (c2a/c2b lhs side);
            # they are DMA-initialized once and never rewritten.
            ftbufs = []
            for b in range(2):
                ft = consts.tile([P, TB * 2 * P], fp8, tag=f"ftbig{b}")
                ft4 = ft[:].rearrange("p (s t m) -> p s t m", s=TB, t=2)
                nc.sync.dma_start(ft4[P - 2 : P, :, 1, :], cones)
                ftbufs.append(ft4)

            m_coll = coll.tile([P, CHUNKS], f32)
            sqa_coll = coll.tile([P, CHUNKS], f32)
            sqb_coll = coll.tile([P, CHUNKS], f32)

            # features viewed as [group, partition, chunk-in-group, d].
            # Partition p takes G consecutive rows (one 8KB contiguous
            # descriptor per partition); chunk->row mapping is permuted,
            # which the order-invariant sum tolerates.
            fview = feats.rearrange("(g p c) d -> g p c d", p=P, c=G)

            for g in range(GROUPS):
                fg = featg_pool.tile([P, G, D], bf16)
                nc.gpsimd.dma_start(fg[:], fview[g])  # SWDGE cast f32->bf16

                for h in range(G // TB):
                    pt = ptrans_pool.tile([D, TB, P], bf16)
                    for j in range(TB):
                        c = h * TB + j
                        nc.tensor.transpose(pt[:, j, :], fg[:, c, :], id_s[:])
                    ft4 = ftbufs[(g * (G // TB) + h) % 2]
                    # evacuate: features (all 128 dims) and squares
                    # (dims 0..125) straight into the fp8 fT buffer
                    nc.scalar.copy(ft4[:, :, 0, :], pt[:])
                    nc.scalar.activation(
                        ft4[0 : P - 2, :, 1, :], pt[0 : P - 2, :, :], Act.Square
                    )

                    for cb in range(TB // TG):
                        px4 = pcross_pool.tile([P, TG, K], f32)
                        for j in range(TG):
                            s = cb * TG + j
                            nc.tensor.matmul(
                                px4[:, j, :],
                                ft4[:, s, :, :],
                                ct_ap,
                                start=True, stop=True,
                                perf_mode=mybir.MatmulPerfMode.DoubleRow,
                                skip_group_check=True,
                            )
                        ib = g * G + h * TB + cb * TG
                        nc.vector.tensor_reduce(
                            out=m_coll[:, ib : ib + TG],
                            in_=px4[:],
                            axis=mybir.AxisListType.X,
                            op=Alu.min,
                        )

                # dims 126/127 enter ||f||^2 via the tail instead of the
                # matmul (their ktile-1 rows carry the c2 constants)
                i0 = g * G
                nc.scalar.activation(
                    sqa_coll[:, i0 : i0 + G], fg[:, :, D - 2], Act.Square
                )
                nc.scalar.activation(
                    sqb_coll[:, i0 : i0 + G], fg[:, :, D - 1], Act.Square
                )

            # tail: sums[p] = sum_i sqrt(m + sq126 + sq127 + c2mean)
            d2t = coll.tile([P, CHUNKS], f32)
            nc.vector.tensor_add(d2t[:], m_coll[:], sqa_coll[:])
            nc.vector.tensor_add(d2t[:], d2t[:], sqb_coll[:])
            dist = coll.tile([P, CHUNKS], f32)
            sums = coll.tile([P, 1], f32)
            nc.scalar.activation(
                dist[:], d2t[:], Act.Sqrt, bias=c2m_s[:], accum_out=sums[:]
            )
            nc.sync.dma_start(out, sums[:])

    nc.compile()
    return nc


def _get_compiled():
    global _compiled
    if _compiled is None:
        _compiled = _build()
    return _compiled


def _make_aux(centers: np.ndarray):
    import ml_dtypes

    e4 = ml_dtypes.float8_e4m3
    cen_bf = centers.astype(ml_dtypes.bfloat16).astype(np.float64)  # [K, D]
    ctneg2_8 = (-2.0 * cen_bf.T).astype(e4)                         # [D, K] fp8
    c_eff = -(ctneg2_8.astype(np.float64)) / 2.0                    # [D, K]
    c2 = (c_eff * c_eff).sum(axis=0)                                # [K]
    c2m = float(c2.mean())
    c2c = c2 - c2m
    c2a = c2c.astype(e4)
    c2b = (c2c - c2a.astype(np.float64)).astype(e4)

    # ctp[p, t*K + k]: t=0 -> -2c[k,p]; t=1 -> p<126: 1.0, p=126: c2a,
    # p=127: c2b
    ctp = np.zeros((P, 2 * K), dtype=e4)
    ctp[:, :K] = ctneg2_8
    ctp[: P - 2, K:] = e4(1.0)
    ctp[P - 2, K:] = c2a
    ctp[P - 1, K:] = c2b

    cones = np.full((2, TB * P), 1.0, dtype=e4)
    ident = np.eye(P, dtype=ml_dtypes.bfloat16)
    c2mean = np.full((P, 1), c2m, dtype=np.float32)
    return ctp, cones, ident, c2mean


def _make_in_maps(features: np.ndarray, centers: np.ndarray):
    ctp, cones, ident, c2mean = _make_aux(centers)
    return [
        {
            "features": features[c * N_PER_CORE : (c + 1) * N_PER_CORE],
            "ctp": ctp,
            "cones": cones,
            "ident": ident,
            "c2mean": c2mean,
        }
        for c in range(N_CORES)
    ]


def kernel(features: np.ndarray, centers: np.ndarray) -> np.ndarray:
    features = np.ascontiguousarray(np.asarray(features, dtype=np.float32))
    centers = np.ascontiguousarray(np.asarray(centers, dtype=np.float32))
    assert features.shape == (N_TOTAL, D) and centers.shape == (K, D)

    from concourse.bass_utils import run_bass_kernel_spmd

    nc = _get_compiled()
    in_maps = _make_in_maps(features, centers)
    res = run_bass_kernel_spmd(nc, in_maps, list(range(N_CORES)))
    total = 0.0
    for r in res.results:
        total += np.sum(r["out"].astype(np.float64))
    return np.float32(total / N_TOTAL)


if __name__ == "__main__":
    rng = np.random.default_rng(0)
    f = rng.standard_normal((N_TOTAL, D), dtype=np.float32)
    c = rng.standard_normal((K, D), dtype=np.float32)
    print(kernel(f, c))
